# revision 7
# baseline (speedup 1.0000x reference)
"""Trainium2 Bass kernel for nn_ChoquetIntegralConstrained.

Computes: sigmoid((x @ w_eff) / weight_sum - thr) where w_eff is built from
(wc, wint) via the constraint transform, x is [16384, 8256] f32.

Strategy: pure data parallel over batch across 8 NeuronCores. Each core
streams its 2048x8256 f32 shard (67.6 MB) over both HWDGE rings (sync +
scalar) at the ~358 GB/s per-core HBM cap; that stream is the roofline.
Each 128-row tile lands as two column halves, one per ring, written into
the same SBUF tile. The dot product runs on the DVE as one
scalar_tensor_tensor (mult, row-sum accumulator) per half — 4128-column
ops amortize the ~0.6 us per-instruction overhead, keeping DVE busy-time
(~160 us) under the DMA stream (~191 us) so compute never paces the
pipeline. The last two tiles arrive as quarter/eighth chunks consumed
arrival-paced to shorten the drain. The weight row is broadcast to 128
partitions via single-pass bf16 PE matmuls against a ones vector (bf16 w
costs ~1.5e-4 output rel err, far inside tolerance); ACT copies PSUM to
SBUF fp32. The tiny constraint transform runs on the host in fp32.
"""

import sys

import numpy as np

sys.path.insert(0, "/opt/trn_rl_repo")

N_CRIT = 128
N_PAIRS = N_CRIT * (N_CRIT - 1) // 2  # 8128
D = N_CRIT + N_PAIRS  # 8256
BATCH = 16384
N_CORES = 8
ROWS_PER_CORE = BATCH // N_CORES  # 2048
P = 128  # SBUF partitions
TILES_PER_CORE = ROWS_PER_CORE // P  # 16
MIN_W = np.float32(1e-07)

HALF = D // 2  # 4128
QTR = D // 4  # 2064
EGT = D // 8  # 1032

_CACHE = {}


def _build_program():
    import concourse.tile as tile
    from concourse import bacc, mybir

    nc = bacc.Bacc(
        "TRN2",
        debug=False,
        target_bir_lowering=False,
        num_devices=N_CORES,
    )
    f32 = mybir.dt.float32
    bf16 = mybir.dt.bfloat16
    x_d = nc.dram_tensor("x", [ROWS_PER_CORE, D], f32, kind="ExternalInput").ap()
    w_d = nc.dram_tensor("w1", [1, D], bf16, kind="ExternalInput").ap()
    c_d = nc.dram_tensor("consts", [P, 2], f32, kind="ExternalInput").ap()
    y_d = nc.dram_tensor("y", [P, TILES_PER_CORE], f32, kind="ExternalOutput").ap()

    N_BODY = TILES_PER_CORE - 2  # tiles 0..13 half-column STTs; 14/15 eighths
    MMCH = 512

    with tile.TileContext(nc) as tc:
        with (
            tc.tile_pool(name="hp", bufs=5) as hp,
            tc.tile_pool(name="tp", bufs=16) as tp,
            tc.tile_pool(name="wp", bufs=1) as wp,
            tc.tile_pool(name="pp", bufs=2, space="PSUM") as pp,
        ):
            # --- weight broadcast: bf16 w row -> 128 partitions via PE ---
            # w_row + consts ride the gpsimd SWDGE ring so the two HWDGE
            # rings carry nothing but the x stream.
            w_row = wp.tile([1, D], bf16)
            nc.gpsimd.dma_start(out=w_row[:], in_=w_d[:])
            ones_t = wp.tile([1, P], bf16)
            nc.gpsimd.memset(ones_t[:], 1.0)
            c_t = wp.tile([P, 2], f32)
            nc.gpsimd.dma_start(out=c_t[:], in_=c_d[:])

            w_h = [
                wp.tile([P, HALF], f32, name="w_h0"),
                wp.tile([P, HALF], f32, name="w_h1"),
            ]

            # Half-tile x buffers; each ring streams one column half of
            # every tile. The first few scalar-ring triggers are emitted
            # before ACT's PSUM->SBUF copy chain so that ring starts
            # streaming immediately.
            x_halves = {}

            def half_dma(t, h):
                x_t = hp.tile([P, HALF], f32, tag="x_h")
                rows = slice(t * P, (t + 1) * P)
                eng = nc.sync if h == 0 else nc.scalar
                eng.dma_start(
                    out=x_t[:], in_=x_d[rows, h * HALF : (h + 1) * HALF]
                )
                x_halves[(t, h)] = x_t

            half_dma(0, 1)
            half_dma(1, 1)
            half_dma(0, 0)
            half_dma(1, 0)

            # Pre-load the Sigmoid ACT table during the ramp so the load is
            # not on the drain critical path.
            warm_t = wp.tile([P, 1], f32)
            nc.scalar.activation(
                out=warm_t[:],
                in_=c_t[:, 0:1],
                func=mybir.ActivationFunctionType.Sigmoid,
                bias=0.0,
                scale=1.0,
            )

            # 18 bf16 matmul chunks (9 per half); ACT copies PSUM -> SBUF.
            for h in range(2):
                off = 0
                while off < HALF:
                    n = min(MMCH, HALF - off)
                    mm = pp.tile([P, MMCH], f32)
                    nc.tensor.matmul(
                        mm[:, 0:n],
                        ones_t[:],
                        w_row[:, h * HALF + off : h * HALF + off + n],
                        start=True,
                        stop=True,
                    )
                    nc.scalar.copy(w_h[h][:, off : off + n], mm[:, 0:n])
                    off += n

            accq_b = wp.tile([P, N_BODY * 2], f32)  # body halves
            accq_t = wp.tile([P, 16], f32)  # tiles 14/15 eighths
            dummy = wp.tile([P, 1], f32)

            def stt(src_ap, w_ap, width, acc_ap):
                nc.vector.scalar_tensor_tensor(
                    out=dummy.broadcast_to((P, width)),
                    in0=src_ap,
                    scalar=1.0,
                    in1=w_ap,
                    op0=mybir.AluOpType.mult,
                    op1=mybir.AluOpType.mult,
                    accum_out=acc_ap,
                )

            # --- body: tiles 0..13, one STT per column half ---
            for t in range(N_BODY):
                if t >= 2:
                    half_dma(t, 0)
                    half_dma(t, 1)
                for h in range(2):
                    stt(
                        x_halves.pop((t, h))[:],
                        w_h[h][:],
                        HALF,
                        accq_b[:, 2 * t + h : 2 * t + h + 1],
                    )

            # --- tiles 14+15 as eighth chunks in a dedicated pool: all 16
            # triggers are dependency-free, so the rings run the tail
            # back-to-back with the body and compute drains arrival-paced.
            dma_eng = (nc.sync, nc.scalar)
            tail = []
            for j in range(16):
                t, e = 14 + j // 8, j % 8
                x_c = tp.tile([P, EGT], f32, tag="xe")
                dma_eng[j % 2].dma_start(
                    out=x_c[:],
                    in_=x_d[t * P : (t + 1) * P, e * EGT : (e + 1) * EGT],
                )
                tail.append(x_c)
            for j in range(16):
                e = j % 8
                h = e // 4
                lo = e * EGT - h * HALF
                stt(
                    tail[j][:],
                    w_h[h][:, lo : lo + EGT],
                    EGT,
                    accq_t[:, j : j + 1],
                )

            # --- finalize: combine partial sums, sigmoid, store ---
            acc_t = wp.tile([P, TILES_PER_CORE], f32)
            nc.vector.tensor_reduce(
                out=acc_t[:, 0:N_BODY],
                in_=accq_b[:].rearrange("p (t q) -> p t q", q=2),
                axis=mybir.AxisListType.X,
                op=mybir.AluOpType.add,
            )
            nc.vector.tensor_reduce(
                out=acc_t[:, N_BODY : N_BODY + 2],
                in_=accq_t[:].rearrange("p (t q) -> p t q", q=8),
                axis=mybir.AxisListType.X,
                op=mybir.AluOpType.add,
            )

            y_t = wp.tile([P, TILES_PER_CORE], f32)
            nc.scalar.activation(
                out=y_t[:],
                in_=acc_t[:],
                func=mybir.ActivationFunctionType.Sigmoid,
                bias=c_t[:, 1:2],
                scale=c_t[:, 0:1],
            )
            nc.sync.dma_start(out=y_d[:], in_=y_t[:])

    nc.compile()
    return nc


def _get_program():
    if "nc" not in _CACHE:
        _CACHE["nc"] = _build_program()
    return _CACHE["nc"]


def _host_weight_prep(wc, wint, thr):
    """Mirror reference._constrained_weights + weight_sum in fp32 numpy."""
    import ml_dtypes

    wc = np.asarray(wc, dtype=np.float32)
    wint = np.asarray(wint, dtype=np.float32)
    wc_eff = np.where(wc < 0, MIN_W, wc)
    ii, jj = np.triu_indices(N_CRIT, k=1)
    lower = np.maximum(-wc_eff[:, ii], -wc_eff[:, jj])
    wint_eff = np.maximum(wint, lower)
    w_eff = np.concatenate([wc_eff, wint_eff], axis=1)  # [1, D]
    wsum = np.float32(wc_eff.sum(dtype=np.float32)) + np.float32(
        wint_eff.sum(dtype=np.float32)
    )
    inv_wsum = np.float32(1.0) / wsum
    neg_thr = -np.float32(np.asarray(thr).reshape(-1)[0])
    return np.ascontiguousarray(w_eff.astype(ml_dtypes.bfloat16)), inv_wsum, neg_thr


def _make_in_maps(x, wc, wint, thr):
    x = np.ascontiguousarray(np.asarray(x, dtype=np.float32))
    w1, inv_wsum, neg_thr = _host_weight_prep(wc, wint, thr)
    consts = np.empty((P, 2), dtype=np.float32)
    consts[:, 0] = inv_wsum
    consts[:, 1] = neg_thr
    return [
        {
            "x": np.ascontiguousarray(x[c * ROWS_PER_CORE : (c + 1) * ROWS_PER_CORE]),
            "w1": w1,
            "consts": consts,
        }
        for c in range(N_CORES)
    ]


def _gather(results):
    # y core tile is [P, TILES]: y[p, t] = batch row t*128 + p within the shard
    parts = [
        np.asarray(results[c]["y"]).T.reshape(ROWS_PER_CORE) for c in range(N_CORES)
    ]
    return np.concatenate(parts).reshape(BATCH, 1).astype(np.float32)


def _run(x, wc, wint, thr, trace=False):
    from concourse import bass_utils

    nc = _get_program()
    in_maps = _make_in_maps(x, wc, wint, thr)
    res = bass_utils.run_bass_kernel_spmd(
        nc, in_maps, core_ids=list(range(N_CORES)), trace=trace
    )
    return _gather(res.results), res


def kernel(x, wc, wint, thr):
    out, _ = _run(x, wc, wint, thr, trace=False)
    return out


# revision 13
# speedup vs baseline: 1.1525x; 1.1525x over previous
"""Trainium2 Bass kernel for nn_ChoquetIntegralConstrained.

Computes: sigmoid((x @ w_eff) / weight_sum - thr) where w_eff is built from
(wc, wint) via the constraint transform, x is [16384, 8256] f32.

Strategy: pure data parallel over batch across 8 NeuronCores. Each core
streams its 2048x8256 f32 shard (67.6 MB) over both HWDGE rings (sync +
scalar) at the ~358 GB/s per-core HBM cap; that stream is the roofline.
Each 128-row tile lands as two column halves, one per ring, written into
the same SBUF tile. The dot product runs on the DVE as one
scalar_tensor_tensor (mult, row-sum accumulator) per half — 4128-column
ops amortize the ~0.6 us per-instruction overhead, keeping DVE busy-time
(~160 us) under the DMA stream (~191 us) so compute never paces the
pipeline. The last two tiles arrive as quarter/eighth chunks consumed
arrival-paced to shorten the drain. The weight row is broadcast to 128
partitions via single-pass bf16 PE matmuls against a ones vector (bf16 w
costs ~1.5e-4 output rel err, far inside tolerance); ACT copies PSUM to
SBUF fp32. The tiny constraint transform runs on the host in fp32.
"""

import sys

import numpy as np

sys.path.insert(0, "/opt/trn_rl_repo")

N_CRIT = 128
N_PAIRS = N_CRIT * (N_CRIT - 1) // 2  # 8128
D = N_CRIT + N_PAIRS  # 8256
BATCH = 16384
N_CORES = 8
ROWS_PER_CORE = BATCH // N_CORES  # 2048
P = 128  # SBUF partitions
TILES_PER_CORE = ROWS_PER_CORE // P  # 16
MIN_W = np.float32(1e-07)

HALF = D // 2  # 4128
QTR = D // 4  # 2064
EGT = D // 8  # 1032

_CACHE = {}


def _build_program():
    import concourse.tile as tile
    from concourse import bacc, mybir

    nc = bacc.Bacc(
        "TRN2",
        debug=False,
        target_bir_lowering=False,
        num_devices=N_CORES,
    )
    f32 = mybir.dt.float32
    bf16 = mybir.dt.bfloat16
    x_d = nc.dram_tensor("x", [ROWS_PER_CORE, D], f32, kind="ExternalInput").ap()
    w_d = nc.dram_tensor("w1", [1, D], bf16, kind="ExternalInput").ap()
    c_d = nc.dram_tensor("consts", [P, 2], f32, kind="ExternalInput").ap()
    y_d = nc.dram_tensor("y", [P, TILES_PER_CORE], f32, kind="ExternalOutput").ap()

    N_BODY = TILES_PER_CORE - 1  # tiles 0..14 half-column STTs; 15 quarters
    MMCH = 512

    with tile.TileContext(nc) as tc:
        with (
            tc.tile_pool(name="hp", bufs=8) as hp,
            tc.tile_pool(name="tp", bufs=4) as tp,
            tc.tile_pool(name="wp", bufs=1) as wp,
            tc.tile_pool(name="pp", bufs=2, space="PSUM") as pp,
        ):
            # --- weight broadcast: bf16 w row -> 128 partitions via PE ---
            # w_row + consts ride the gpsimd SWDGE ring so the two HWDGE
            # rings carry nothing but the x stream. w_row borrows an x-half
            # pool slot (exactly 16.5KB); it is released once the broadcast
            # matmuls have read it.
            w_row = hp.tile([1, D], bf16, tag="x_h")
            nc.gpsimd.dma_start(out=w_row[:], in_=w_d[:])
            ones_t = wp.tile([1, P], bf16)
            nc.gpsimd.memset(ones_t[:], 1.0)
            c_t = wp.tile([P, 2], f32)
            nc.gpsimd.dma_start(out=c_t[:], in_=c_d[:])

            w_h = [
                wp.tile([P, HALF], f32, name="w_h0"),
                wp.tile([P, HALF], f32, name="w_h1"),
            ]

            # Half-tile x buffers; each ring streams one column half of
            # every tile. The pool is 8 slots deep (4 tiles ~ 48us of
            # stream) so the trigger -> DGE -> transfer -> completion-sem ->
            # STT -> next-trigger latency loop never throttles the rings.
            # The first few triggers on each ring are emitted before ACT's
            # PSUM->SBUF copy chain so both rings stream immediately.
            x_halves = {}

            def half_dma(t, h):
                x_t = hp.tile([P, HALF], f32, tag="x_h")
                rows = slice(t * P, (t + 1) * P)
                eng = nc.sync if h == 0 else nc.scalar
                eng.dma_start(
                    out=x_t[:], in_=x_d[rows, h * HALF : (h + 1) * HALF]
                )
                x_halves[(t, h)] = x_t

            for t in range(3):
                half_dma(t, 1)
                half_dma(t, 0)

            # Pre-load the Sigmoid ACT table during the ramp so the load is
            # not on the drain critical path.
            warm_t = wp.tile([P, 1], f32)
            nc.scalar.activation(
                out=warm_t[:],
                in_=c_t[:, 0:1],
                func=mybir.ActivationFunctionType.Sigmoid,
                bias=0.0,
                scale=1.0,
            )

            # 18 bf16 matmul chunks (9 per half); ACT copies PSUM -> SBUF.
            for h in range(2):
                off = 0
                while off < HALF:
                    n = min(MMCH, HALF - off)
                    mm = pp.tile([P, MMCH], f32)
                    nc.tensor.matmul(
                        mm[:, 0:n],
                        ones_t[:],
                        w_row[:, h * HALF + off : h * HALF + off + n],
                        start=True,
                        stop=True,
                    )
                    nc.scalar.copy(w_h[h][:, off : off + n], mm[:, 0:n])
                    off += n

            accq_b = wp.tile([P, N_BODY * 2], f32)  # body halves
            accq_t = wp.tile([P, 4], f32)  # tile 15 quarters
            dummy = wp.tile([P, 1], f32)

            def stt(src_ap, w_ap, width, acc_ap):
                nc.vector.scalar_tensor_tensor(
                    out=dummy.broadcast_to((P, width)),
                    in0=src_ap,
                    scalar=1.0,
                    in1=w_ap,
                    op0=mybir.AluOpType.mult,
                    op1=mybir.AluOpType.mult,
                    accum_out=acc_ap,
                )

            # --- body: tiles 0..14, one STT per column half ---
            acc_t = wp.tile([P, TILES_PER_CORE], f32)
            for t in range(N_BODY):
                nxt = t + 3
                if nxt < N_BODY:
                    half_dma(nxt, 0)
                    half_dma(nxt, 1)
                for h in range(2):
                    stt(
                        x_halves.pop((t, h))[:],
                        w_h[h][:],
                        HALF,
                        accq_b[:, 2 * t + h : 2 * t + h + 1],
                    )
                if t == N_BODY - 1:
                    # Body combine runs hidden while tile 15 streams.
                    nc.vector.tensor_reduce(
                        out=acc_t[:, 0:N_BODY],
                        in_=accq_b[:].rearrange("p (t q) -> p t q", q=2),
                        axis=mybir.AxisListType.X,
                        op=mybir.AluOpType.add,
                    )

            # --- tile 15 as quarter chunks in a dedicated pool: all 4
            # triggers are dependency-free, so the rings run the tail
            # back-to-back with the body and compute drains arrival-paced.
            dma_eng = (nc.sync, nc.scalar)
            tail = []
            for q in range(4):
                x_c = tp.tile([P, QTR], f32, tag="xq")
                dma_eng[q % 2].dma_start(
                    out=x_c[:],
                    in_=x_d[15 * P : 16 * P, q * QTR : (q + 1) * QTR],
                )
                tail.append(x_c)
            for q in range(4):
                h = q // 2
                lo = (q % 2) * QTR
                stt(
                    tail[q][:],
                    w_h[h][:, lo : lo + QTR],
                    QTR,
                    accq_t[:, q : q + 1],
                )

            # --- finalize: combine tile-15 sums, sigmoid, store ---
            nc.vector.tensor_reduce(
                out=acc_t[:, N_BODY : N_BODY + 1],
                in_=accq_t[:].rearrange("p (t q) -> p t q", q=4),
                axis=mybir.AxisListType.X,
                op=mybir.AluOpType.add,
            )

            y_t = wp.tile([P, TILES_PER_CORE], f32)
            nc.scalar.activation(
                out=y_t[:],
                in_=acc_t[:],
                func=mybir.ActivationFunctionType.Sigmoid,
                bias=c_t[:, 1:2],
                scale=c_t[:, 0:1],
            )
            nc.sync.dma_start(out=y_d[:], in_=y_t[:])

    nc.compile()
    return nc


def _get_program():
    if "nc" not in _CACHE:
        _CACHE["nc"] = _build_program()
    return _CACHE["nc"]


def _host_weight_prep(wc, wint, thr):
    """Mirror reference._constrained_weights + weight_sum in fp32 numpy."""
    import ml_dtypes

    wc = np.asarray(wc, dtype=np.float32)
    wint = np.asarray(wint, dtype=np.float32)
    wc_eff = np.where(wc < 0, MIN_W, wc)
    ii, jj = np.triu_indices(N_CRIT, k=1)
    lower = np.maximum(-wc_eff[:, ii], -wc_eff[:, jj])
    wint_eff = np.maximum(wint, lower)
    w_eff = np.concatenate([wc_eff, wint_eff], axis=1)  # [1, D]
    wsum = np.float32(wc_eff.sum(dtype=np.float32)) + np.float32(
        wint_eff.sum(dtype=np.float32)
    )
    inv_wsum = np.float32(1.0) / wsum
    neg_thr = -np.float32(np.asarray(thr).reshape(-1)[0])
    return np.ascontiguousarray(w_eff.astype(ml_dtypes.bfloat16)), inv_wsum, neg_thr


def _make_in_maps(x, wc, wint, thr):
    x = np.ascontiguousarray(np.asarray(x, dtype=np.float32))
    w1, inv_wsum, neg_thr = _host_weight_prep(wc, wint, thr)
    consts = np.empty((P, 2), dtype=np.float32)
    consts[:, 0] = inv_wsum
    consts[:, 1] = neg_thr
    return [
        {
            "x": np.ascontiguousarray(x[c * ROWS_PER_CORE : (c + 1) * ROWS_PER_CORE]),
            "w1": w1,
            "consts": consts,
        }
        for c in range(N_CORES)
    ]


def _gather(results):
    # y core tile is [P, TILES]: y[p, t] = batch row t*128 + p within the shard
    parts = [
        np.asarray(results[c]["y"]).T.reshape(ROWS_PER_CORE) for c in range(N_CORES)
    ]
    return np.concatenate(parts).reshape(BATCH, 1).astype(np.float32)


def _run(x, wc, wint, thr, trace=False):
    from concourse import bass_utils

    nc = _get_program()
    in_maps = _make_in_maps(x, wc, wint, thr)
    res = bass_utils.run_bass_kernel_spmd(
        nc, in_maps, core_ids=list(range(N_CORES)), trace=trace
    )
    return _gather(res.results), res


def kernel(x, wc, wint, thr):
    out, _ = _run(x, wc, wint, thr, trace=False)
    return out
